# revision 26
# baseline (speedup 1.0000x reference)
# Trainium2 Bass kernel for GQA attention block (B=8, T=512, C=2048, 16 q heads,
# 4 kv heads, head_dim=128, RoPE, causal mask, output projection).
#
# Strategy: data parallel across the 8 NeuronCores — core i handles batch
# element i with the full weight set (no collectives). Per core everything is
# computed in a transposed layout:
#   qT/kT [d, t]  = W[:, d-tile].T-contract  (lhsT = weight tile, rhs = xT)
#   scoresT [s,t] = kT-slice.T @ qT          (softmax normalizer along the
#                                             partition dim via a ones-matmul
#                                             that broadcasts the sum to all
#                                             128 partitions for free)
#   outT [d, t]   = v-slice.T @ (mask*exp(scoresT))  (unnormalized)
#   y [t, e]      = outT-slice.T @ wo-tile   (normalized outT)
# RoPE rotate-half is a signed 128x128 permutation matmul + 2 muls + 1 add.
# Matmul operands are fp16 by default (full PE rate, half the HBM bytes of
# fp32, DVE 2x modes; fp32 PSUM accumulation throughout). When the runtime
# mask is exactly causal, the scores/AV/normalizer matmuls restrict their
# moving dim to the visible t-range and only the diagonal 128x128 block gets
# mask-multiplied; any other mask falls back to a general masked build.

import os
import sys

import numpy as np

for _p in (
    "/root/.axon_site",
    "/root/.axon_site/_ro/trn_rl_repo",
    "/root/.axon_site/_ro/pypackages",
    "/opt/trn_rl_repo",
):
    if os.path.isdir(_p) and _p not in sys.path:
        sys.path.append(_p)

import concourse.bass as bass  # noqa: E402
import concourse.mybir as mybir  # noqa: E402
import concourse.tile as tile  # noqa: E402
from concourse import bacc  # noqa: E402
from concourse.bass_utils import run_bass_kernel_spmd  # noqa: E402

F32 = mybir.dt.float32
F32R = mybir.dt.float32r
F16 = mybir.dt.float16
AF = mybir.ActivationFunctionType

B, T, C = 8, 512, 2048
HD, NH, NKV = 128, 16, 4
CT = C // 128  # 16 contraction tiles
TT = T // 128  # 4 t/s tiles
EG = C // 512  # 4 output column groups
REP = NH // NKV
SCALE = float(HD) ** -0.5
N_CORES = 8

# Matmul operand dtype. fp16 (default): full PE rate, half the DMA bytes,
# DVE 2x modes; fp32r: full-rate reduced-precision fp32; fp32: exact, 4x
# slower on the PE.
_DT_ENV = os.environ.get("ATTN_DTYPE", "fp16")
MM_DT = {"fp16": F16, "fp32r": F32R, "fp32": F32}[_DT_ENV]
MM_NP = {"fp16": np.float16, "fp32r": np.float32, "fp32": np.float32}[_DT_ENV]


def _s(i, n):
    return slice(i * n, (i + 1) * n)




def _emit(tc, xT, wq, wk, wv, wo, cosT, sinT, maskT, ones, y, causal):
    nc = tc.nc
    mm = nc.tensor.matmul

    with (
        tc.tile_pool(name="consts", bufs=1) as consts,
        tc.tile_pool(name="streams", bufs=2) as streams,
        tc.tile_pool(name="work", bufs=2) as work,
        tc.tile_pool(name="ps", bufs=1, space="PSUM") as ps,
    ):
        cosT_sb = consts.tile([HD, T], MM_DT)
        sinT_sb = consts.tile([HD, T], MM_DT)
        maskT_sb = consts.tile([128, TT, T], MM_DT)
        ones_sb = consts.tile([128, 128], MM_DT)
        xT_sb = consts.tile([128, CT, T], MM_DT)
        kT_sb = consts.tile([HD, NKV, T], MM_DT)
        v_sb = consts.tile([128, TT, 4 * HD], MM_DT)
        aout_sb = consts.tile([HD, NH, T], MM_DT)

        # wq head-slice prefetch (scalar ring), depth = bufs
        wqh_tiles = {}
        _prefetched = []

        def wqh_dma(h, eng=None):
            if h >= NH:
                return
            wqh = streams.tile([128, CT, HD], MM_DT, tag="wqh", bufs=4, name=f"wqh{h}")
            (eng or nc.scalar).dma_start(
                wqh[:], wq[:, _s(h, HD)].rearrange("(ct p) d -> p ct d", p=128)
            )
            wqh_tiles[h] = wqh

        # ---- rope helper: psrc (PSUM, [d, t] f32) -> out_slice (SBUF) ----
        # rotate-half via partition-shifted PSUM reads; sinT_sb rows 0:64 are
        # pre-negated on the host, so no rotation matmul is needed.
        def rope(psrc, out_slice, tag):
            qrot = work.tile([HD, T], MM_DT, tag="trot", name=f"qrot_{tag}")
            nc.scalar.copy(qrot[0:64, :], psrc[64:128, :])
            nc.scalar.copy(qrot[64:128, :], psrc[0:64, :])
            tcos = work.tile([HD, T], MM_DT, tag="tcos", name=f"tcos_{tag}")
            nc.vector.tensor_mul(tcos[:], psrc, cosT_sb[:])
            nc.vector.tensor_mul(qrot[:], qrot[:], sinT_sb[:])
            nc.vector.tensor_add(out_slice, tcos[:], qrot[:])

        # first xT tiles via the scalar HWDGE ring (low latency for the very
        # first matmuls); the rest + constants via SWDGE, keeping both HWDGE
        # rings free for the weight streams and the ACT queue for compute
        for ct in range(4):
            nc.scalar.dma_start(xT_sb[:, ct, :], xT[_s(ct, 128), :])
        for ct in range(4, CT):
            nc.gpsimd.dma_start(xT_sb[:, ct, :], xT[_s(ct, 128), :])
        nc.gpsimd.dma_start(cosT_sb[:], cosT)
        nc.gpsimd.dma_start(sinT_sb[:], sinT)
        nc.gpsimd.dma_start(ones_sb[:], ones)
        nc.gpsimd.dma_start(
            maskT_sb[:], maskT.rearrange("(st p) t -> p st t", p=128)
        )

        # ---- phase A: v projection, then k projection (rope-k runs inside
        # the phase-B pipeline, overlapping the first head projections) ----
        vp = ps.tile([128, TT, 4 * HD], F32, tag="big", bufs=1)
        for nch, (c0, ncs) in enumerate([(0, 1), (1, 1), (2, 2), (4, 4), (8, 4), (12, 4)]):
            vwt = streams.tile(
                [128, 4, 4 * HD], MM_DT, tag="wkv4", bufs=8, name=f"vwt{nch}"
            )
            nc.sync.dma_start(
                vwt[:, :ncs, :],
                wv[c0 * 128 : (c0 + ncs) * 128, :].rearrange(
                    "(c p) d -> p c d", p=128
                ),
            )
            for ci in range(ncs):
                ct = c0 + ci
                for i in range(TT):
                    mm(
                        vp[:, i, :],
                        xT_sb[:, ct, _s(i, 128)],
                        vwt[:, ci, :],
                        start=(ct == 0),
                        stop=(ct == CT - 1),
                    )
        

        for i in range(TT):
            nc.vector.tensor_copy(v_sb[:, i, :], vp[:, i, :])

        _kp_tags = (("qp", 2), ("qp", 2), ("av", 1), ("lsum", 1))
        kps = [
            ps.tile([HD, T], F32, tag=t, bufs=bf, name=f"kp{j}")
            for j, (t, bf) in enumerate(_kp_tags)
        ]
        kwts = []
        for cg in range(4):
            kwt = streams.tile(
                [128, 4, 4 * HD], MM_DT, tag="wkv4", bufs=8, name=f"kwt{cg}"
            )
            nc.sync.dma_start(
                kwt[:], wk[_s(cg, 512), :].rearrange("(c p) d -> p c d", p=128)
            )
            kwts.append(kwt)
        for j in range(NKV):
            for cg in range(4):
                for ci in range(4):
                    ct = cg * 4 + ci
                    mm(
                        kps[j][:],
                        kwts[cg][:, ci, _s(j, HD)],
                        xT_sb[:, ct, :],
                        start=(ct == 0),
                        stop=(ct == CT - 1),
                    )
        for h in range(3):
            wqh_dma(h, nc.sync)

        rope(kps[0][:], kT_sb[:, 0, :], "k0")
        rope(kps[1][:], kT_sb[:, 1, :], "k1")

        # ---- phase B: per q head, software pipelined ----
        state = {}

        def stage_a(h):  # projection matmuls into psum
            qp = ps.tile([HD, T], F32, tag="qp", bufs=2, name=f"qp{h}")
            wqh = wqh_tiles.pop(h)
            for ct in range(CT):
                mm(
                    qp[:],
                    wqh[:, ct, :],
                    xT_sb[:, ct, :],
                    start=(ct == 0),
                    stop=(ct == CT - 1),
                )
            wqh_dma(h + 3)
            state[h] = {"qp": qp}

        def stage_b(h):  # rope (straight from psum) -> qT
            qT = work.tile([HD, T], MM_DT, tag="qT", bufs=2, name=f"qT{h}")
            rope(state[h]["qp"][:], qT[:], f"q{h}")
            state[h]["qT"] = qT

        def stage_c1(h):  # scoresT matmuls, exp, mask (per s-tile bank)
            j = h // REP
            sT = ps.tile([128, TT, T], F32, tag="big", bufs=1, name=f"sT{h}")
            qT = state[h]["qT"]
            for i in range(TT):
                lo = 128 * i if causal else 0
                mm(
                    sT[:, i, lo:],
                    kT_sb[:, j, _s(i, 128)],
                    qT[:, lo:],
                    start=True,
                    stop=True,
                )
            expm = work.tile([128, TT, T], MM_DT, tag="expm", bufs=2, name=f"expm{h}")
            for i in range(TT):
                lo = 128 * i if causal else 0
                nc.scalar.activation(
                    expm[:, i, lo:], sT[:, i, lo:], AF.Exp, scale=SCALE
                )
                if causal:
                    # only the diagonal 128x128 block is partially masked;
                    # t < lo is never read downstream, t >= lo+128 is fully
                    # visible
                    nc.vector.tensor_mul(
                        expm[:, i, lo : lo + 128],
                        expm[:, i, lo : lo + 128],
                        maskT_sb[:, i, lo : lo + 128],
                    )
                else:
                    nc.vector.tensor_mul(
                        expm[:, i, :], expm[:, i, :], maskT_sb[:, i, :]
                    )
            state[h]["expm"] = expm

        def stage_c2(h):  # AV + normalizer matmuls, reciprocal, scale into aout
            j = h // REP
            expm = state[h]["expm"]
            avp = ps.tile([HD, T], F32, tag="av", bufs=1, name=f"avp{h}")
            for i in range(TT):
                lo = 128 * i if causal else 0
                mm(
                    avp[:, lo:],
                    v_sb[:, i, _s(j, HD)],
                    expm[:, i, lo:],
                    start=(i == 0),
                    stop=(i == TT - 1),
                )
            lp = ps.tile([128, T], F32, tag="lsum", bufs=1, name=f"lp{h}")
            for i in range(TT):
                lo = 128 * i if causal else 0
                mm(
                    lp[:, lo:],
                    ones_sb[:],
                    expm[:, i, lo:],
                    start=(i == 0),
                    stop=(i == TT - 1),
                )
            recip = work.tile([HD, T], F32, tag="recip", name=f"recip{h}")
            nc.vector.reciprocal_approx_fast(recip[:], lp[:HD, :])
            nc.vector.tensor_mul(aout_sb[:, h, :], avp[:], recip[:])
            del state[h]

        for it in range(NH + 3):
            if it < NH:
                stage_a(it)
            if it < 2:
                rope(kps[it + 2][:], kT_sb[:, it + 2, :], f"k{it + 2}")
            if 0 <= it - 3 < NH:
                stage_c2(it - 3)
            if it < NH:
                stage_b(it)
            if 0 <= it - 2 < NH:
                stage_c1(it - 2)

        # ---- phase C: output projection y = aout.T @ wo ----
        for eg in range(EG):
            if eg % 2 == 0:
                yp = ps.tile([128, TT, 512], F32, tag="big", bufs=1, name=f"yp{eg}")
                yslices = [yp[:, i, :] for i in range(TT)]
            else:
                yts = [
                    ps.tile([128, 512], F32, tag=t, bufs=bf, name=f"yp{eg}_{i}")
                    for i, (t, bf) in enumerate(
                        (("qp", 2), ("qp", 2), ("av", 1), ("lsum", 1))
                    )
                ]
                yslices = [t[:] for t in yts]
            for fg in range(4):
                wot = streams.tile(
                    [128, 4, 512], MM_DT, tag="wkv4", bufs=8, name=f"wot{eg}_{fg}"
                )
                (nc.sync if fg % 2 == 0 else nc.scalar).dma_start(
                    wot[:],
                    wo[_s(fg, 512), _s(eg, 512)].rearrange("(c p) e -> p c e", p=128),
                )
                for ci in range(4):
                    ft = fg * 4 + ci
                    for i in range(TT):
                        mm(
                            yslices[i],
                            aout_sb[:, ft, _s(i, 128)],
                            wot[:, ci, :],
                            start=(ft == 0),
                            stop=(ft == CT - 1),
                        )
            for i in range(TT):
                ysb_i = work.tile(
                    [128, 512], F32, tag="ysb1", bufs=4, name=f"ysb{eg}_{i}"
                )
                if i % 2 == 0:
                    nc.scalar.copy(ysb_i[:], yslices[i])
                else:
                    nc.vector.tensor_copy(ysb_i[:], yslices[i])
                (nc.scalar if i % 2 == 0 else nc.sync).dma_start(y[_s(i, 128), _s(eg, 512)], ysb_i[:])


def build(causal=False):
    nc = bacc.Bacc(
        "TRN2",
        target_bir_lowering=False,
        debug=False,
        enable_asserts=False,
        num_devices=N_CORES,
    )
    xT = nc.dram_tensor("xT", [C, T], MM_DT, kind="ExternalInput").ap()
    wq = nc.dram_tensor("wq", [C, C], MM_DT, kind="ExternalInput").ap()
    wk = nc.dram_tensor("wk", [C, NKV * HD], MM_DT, kind="ExternalInput").ap()
    wv = nc.dram_tensor("wv", [C, NKV * HD], MM_DT, kind="ExternalInput").ap()
    wo = nc.dram_tensor("wo", [C, C], MM_DT, kind="ExternalInput").ap()
    cosT = nc.dram_tensor("cosT", [HD, T], MM_DT, kind="ExternalInput").ap()
    sinT = nc.dram_tensor("sinT", [HD, T], MM_DT, kind="ExternalInput").ap()
    maskT = nc.dram_tensor("maskT", [T, T], MM_DT, kind="ExternalInput").ap()
    ones = nc.dram_tensor("ones", [128, 128], MM_DT, kind="ExternalInput").ap()
    y = nc.dram_tensor("y", [T, C], F32, kind="ExternalOutput").ap()

    with tile.TileContext(nc) as tc:
        _emit(tc, xT, wq, wk, wv, wo, cosT, sinT, maskT, ones, y, causal)
    nc.compile()
    return nc


_NC = {}


def _get_nc(causal):
    if causal not in _NC:
        _NC[causal] = build(causal)
    return _NC[causal]


def _is_causal(mask):
    return bool(np.array_equal(mask, np.tril(np.ones((T, T), dtype=bool))))


def host_tables():
    """cos/sin tables (transposed) and the signed rotate-half matrix."""
    inv = 1.0 / (10000.0 ** (np.arange(0, HD, 2, dtype=np.float32) / HD))
    t = np.arange(T, dtype=np.float32)
    freqs = np.outer(t, inv)  # [T, HD/2]
    emb = np.concatenate([freqs, freqs], axis=-1)  # [T, HD]
    cosT = np.ascontiguousarray(np.cos(emb).T, dtype=np.float32)
    sinT = np.ascontiguousarray(np.sin(emb).T, dtype=np.float32)
    # rotate-half signs baked in: rows d<64 multiply the shifted-down half
    # with a minus sign (q'[d] = q[d]cos - q[d+64]sin for d<64)
    sinT[: HD // 2] *= -1.0
    return cosT, sinT


def make_in_maps(inputs):
    x = np.asarray(inputs["x"], dtype=np.float32)
    mask = np.asarray(inputs["mask"]).reshape(T, T)
    cosT, sinT = host_tables()
    shared = {
        "wq": np.ascontiguousarray(np.asarray(inputs["wq"]).astype(MM_NP)),
        "wk": np.ascontiguousarray(np.asarray(inputs["wk"]).astype(MM_NP)),
        "wv": np.ascontiguousarray(np.asarray(inputs["wv"]).astype(MM_NP)),
        "wo": np.ascontiguousarray(np.asarray(inputs["wo"]).astype(MM_NP)),
        "cosT": cosT.astype(MM_NP),
        "sinT": sinT.astype(MM_NP),
        "maskT": np.ascontiguousarray(mask.T).astype(MM_NP),  # [s, t]
        "ones": np.ones((128, 128), dtype=MM_NP),
    }
    return [
        {"xT": np.ascontiguousarray(x[b].T).astype(MM_NP), **shared}
        for b in range(N_CORES)
    ]


def run(inputs, **kw):
    mask = np.asarray(inputs["mask"]).reshape(T, T)
    nc = _get_nc(_is_causal(mask))
    in_maps = make_in_maps(inputs)
    res = run_bass_kernel_spmd(nc, in_maps, core_ids=list(range(N_CORES)), **kw)
    out = np.stack([r["y"] for r in res.results], axis=0)
    return out, res


def kernel(**inputs) -> np.ndarray:
    out, _ = run(inputs)
    return out
